# revision 11
# baseline (speedup 1.0000x reference)
"""Cross-attention block (B=16, N=4096 queries, M=77 keys, 8 heads x 64) on 8 trn2 cores.

Sharding: data-parallel over batch; each core gets 2 batches, full weights.

Per-core dataflow (matmuls bf16 in / fp32 psum):
  x -> bf16 staging copy in DRAM (gpsimd cast DMA), per 512-token chunk
  xbar-transpose-loaded as xT [feat, tok].
  qT = Wq.T @ xT                   (weight-stationary)
  per head h: sT = kT_h.T @ qT_h -> exp(sT/8) -> E[77, H, tok]
  denominators: 8 indicator-matmuls accumulate colsum(E_h) into psum [8, tok],
  reciprocal_approx_fast, bounce through DRAM to broadcast across partitions.
  per head-pair: O.T = v_h.T @ E_h into psum halves; aT = O.T * recip (DVE)
  out = aT.T @ Wo + bo             (aT chunks stationary -> token-major out)
"""

import numpy as np

import concourse.bass as bass
import concourse.mybir as mybir
import concourse.tile as tile
from concourse import bacc
from concourse._compat import with_exitstack
from concourse.bass_utils import run_bass_kernel_spmd
from concourse.masks import make_identity
from contextlib import ExitStack

N_CORES = 8
B, N, FEAT, CD = 16, 4096, 512, 768
M = 77          # cond tokens
H, DH = 8, 64
DA = H * DH     # 512
BP = B // N_CORES   # batches per core
TC = 512            # token chunk
NT = N // TC        # chunks per batch
SUB = TC // 128     # 128-token subtiles per chunk
KC = FEAT // 128    # x feature chunks
CC = CD // 128      # cond feature chunks
MC = DA // 128      # d_attn chunks
HPAIRS = H // 2

F32 = mybir.dt.float32
BF16 = mybir.dt.bfloat16
EXP = mybir.ActivationFunctionType.Exp


@with_exitstack
def _body(ctx: ExitStack, tc: tile.TileContext, x, x_bf, cond, Wq, Wk, Wv, Wo, bo, out):
    nc = tc.nc

    wpool = ctx.enter_context(tc.tile_pool(name="wpool", bufs=1))
    Wq_bf = wpool.tile([128, KC, DA], BF16, tag="wq")
    Wk_bf = wpool.tile([128, CC, DA], BF16, tag="wk")
    Wv_bf = wpool.tile([128, CC, DA], BF16, tag="wv")
    Wo_bf = wpool.tile([128, MC, FEAT], BF16, tag="wo")
    bo_bc = wpool.tile([128, FEAT], F32, tag="bo")
    ident = wpool.tile([128, 128], F32, tag="ident")
    # 0/1 picker: col 8 is ones; colpick[:, 8-h : 16-h] selects head h
    colpick = wpool.tile([128, 17], BF16, tag="colpick")

    for k in range(KC):
        nc.gpsimd.dma_start(out=Wq_bf[:, k, :], in_=Wq[128 * k : 128 * (k + 1), :])
    for c in range(CC):
        nc.gpsimd.dma_start(out=Wk_bf[:, c, :], in_=Wk[128 * c : 128 * (c + 1), :])
        nc.gpsimd.dma_start(out=Wv_bf[:, c, :], in_=Wv[128 * c : 128 * (c + 1), :])
    for m in range(MC):
        nc.gpsimd.dma_start(out=Wo_bf[:, m, :], in_=Wo[128 * m : 128 * (m + 1), :])
    bo_bcast_ap = bass.AP(tensor=bo.tensor, offset=bo.offset, ap=[[0, 128], *bo.ap])
    nc.gpsimd.dma_start(out=bo_bc[:, :], in_=bo_bcast_ap)
    make_identity(nc, ident)
    nc.gpsimd.memset(colpick[:, :], 0.0)
    nc.gpsimd.memset(colpick[:, 8:9], 1.0)

    # bf16 staging copy of x (transpose-loads below need a 2-byte dtype)
    for b in range(BP):
        for q in range(4):
            nc.gpsimd.dma_start(
                out=x_bf[b, 1024 * q : 1024 * (q + 1), :],
                in_=x[b, 1024 * q : 1024 * (q + 1), :],
            )

    bpool = ctx.enter_context(tc.tile_pool(name="bpool", bufs=2))
    tpool = ctx.enter_context(tc.tile_pool(name="tpool", bufs=3))
    qpool = ctx.enter_context(tc.tile_pool(name="qpool", bufs=3))
    epool = ctx.enter_context(tc.tile_pool(name="epool", bufs=3))
    rpool = ctx.enter_context(tc.tile_pool(name="rpool", bufs=6))
    apool = ctx.enter_context(tc.tile_pool(name="apool", bufs=3))
    opool = ctx.enter_context(tc.tile_pool(name="opool", bufs=4))

    dpool = ctx.enter_context(tc.tile_pool(name="dpool", bufs=2, space="DRAM"))

    psq = ctx.enter_context(tc.tile_pool(name="psq", bufs=2, space="PSUM"))
    pss = ctx.enter_context(tc.tile_pool(name="pss", bufs=3, space="PSUM"))
    psu = ctx.enter_context(tc.tile_pool(name="psu", bufs=2, space="PSUM"))
    psm = ctx.enter_context(tc.tile_pool(name="psm", bufs=1, space="PSUM"))

    for b in range(BP):
        # cond[b] -> cond.T (PE transpose) -> K/V projections
        cond_sb = bpool.tile([128, CD], F32, tag="cond")
        nc.sync.dma_start(out=cond_sb[:M, :], in_=cond[b, :, :])
        condT = bpool.tile([128, CC, M], BF16, tag="condT")
        for c in range(CC):
            ps = pss.tile([128, TC], F32, tag="pss")
            nc.tensor.matmul(
                ps[:128, :M],
                cond_sb[:M, 128 * c : 128 * (c + 1)],
                ident[:M, :M],
                is_transpose=True,
            )
            nc.scalar.copy(condT[:, c, :], ps[:128, :M])

        # kT[d_attn, M] = Wk.T @ cond.T
        kT = bpool.tile([128, MC, M], BF16, tag="kT")
        for m in range(MC):
            pk = psq.tile([128, TC], F32, tag="psq")
            for c in range(CC):
                nc.tensor.matmul(
                    pk[:, :M],
                    Wk_bf[:, c, 128 * m : 128 * (m + 1)],
                    condT[:, c, :],
                    start=(c == 0),
                    stop=(c == CC - 1),
                )
            nc.scalar.copy(kT[:, m, :], pk[:, :M])

        # v[M, d_attn] = cond @ Wv  (cond.T is the stationary operand)
        pv = pss.tile([128, TC], F32, tag="pss")
        for c in range(CC):
            nc.tensor.matmul(
                pv[:M, :],
                condT[:, c, :],
                Wv_bf[:, c, :],
                start=(c == 0),
                stop=(c == CC - 1),
            )
        v_bf = bpool.tile([128, DA], BF16, tag="v")
        nc.scalar.copy(v_bf[:M, :], pv[:M, :])

        for t in range(NT):
            tok0 = t * TC
            # xT[feat, tok] via xbar transpose straight from the bf16 staging copy
            xT = tpool.tile([128, KC, TC], BF16, tag="xT")
            for k in range(KC):
                nc.sync.dma_start(
                    out=xT[:, k, :],
                    in_=x_bf[b, tok0 : tok0 + TC, 128 * k : 128 * (k + 1)],
                    transpose=True,
                )

            # qT[d_attn, tok] = Wq.T @ xT
            qT = qpool.tile([128, MC, TC], BF16, tag="qT")
            for m in range(MC):
                pq = psq.tile([128, TC], F32, tag="psq")
                for k in range(KC):
                    nc.tensor.matmul(
                        pq,
                        Wq_bf[:, k, 128 * m : 128 * (m + 1)],
                        xT[:, k, :],
                        start=(k == 0),
                        stop=(k == KC - 1),
                    )
                nc.scalar.copy(qT[:, m, :], pq)

            # scores + exp per head; also accumulate per-head colsums on PE
            E = epool.tile([128, H, TC], BF16, tag="E")
            sm = psm.tile([8, TC], F32, tag="psm")
            for h in range(H):
                hp, r = h // 2, 64 * (h % 2)
                ps = pss.tile([128, TC], F32, tag="pss")
                nc.tensor.matmul(
                    ps[:M, :],
                    kT[r : r + 64, hp, :],
                    qT[r : r + 64, hp, :],
                    start=True,
                    stop=True,
                )
                nc.scalar.activation(E[:M, h, :], ps[:M, :], func=EXP, scale=DH**-0.5)
                nc.tensor.matmul(
                    sm,
                    colpick[:M, 8 - h : 16 - h],
                    E[:M, h, :],
                    start=(h == 0),
                    stop=(h == H - 1),
                )

            # 1/sums, bounced through DRAM to broadcast rows across partitions
            r8 = rpool.tile([8, TC], F32, tag="r8")
            nc.vector.reciprocal_approx_fast(out=r8[:8, :], in_=sm[:8, :])
            r8d = dpool.tile([8, TC], F32, tag="r8d")
            nc.sync.dma_start(out=r8d[:, :], in_=r8[:8, :])
            rss = []
            for hp in range(HPAIRS):
                rs = rpool.tile([128, TC], F32, tag="rs")
                bcast_src = r8d[2 * hp : 2 * hp + 2, :]
                bcast_ap = bass.AP(
                    tensor=bcast_src.tensor,
                    offset=bcast_src.offset,
                    ap=[bcast_src.ap[0], [0, 64], *bcast_src.ap[1:]],
                )
                nc.sync.dma_start(out=rs[:, :], in_=bcast_ap)
                rss.append(rs)

            # attn @ v, normalized at copyback: aT[d_attn, tok]
            aT = apool.tile([128, MC, TC], BF16, tag="aT")
            for hp in range(HPAIRS):
                po = pss.tile([128, TC], F32, tag="pss")
                nc.tensor.matmul(
                    po[0:64, :],
                    v_bf[:M, 128 * hp : 128 * hp + 64],
                    E[:M, 2 * hp, :],
                    start=True,
                    stop=True,
                )
                nc.tensor.matmul(
                    po[64:128, :],
                    v_bf[:M, 128 * hp + 64 : 128 * (hp + 1)],
                    E[:M, 2 * hp + 1, :],
                    start=True,
                    stop=True,
                )
                nc.vector.tensor_mul(aT[:, hp, :], po[:, :], rss[hp][:, :])

            # out = aT.T @ Wo + bo  (aT chunks stationary -> token-major psum)
            for s in range(SUB):
                pu = psu.tile([128, FEAT], F32, tag="psu")
                for m in range(MC):
                    nc.tensor.matmul(
                        pu,
                        aT[:, m, 128 * s : 128 * (s + 1)],
                        Wo_bf[:, m, :],
                        start=(m == 0),
                        stop=(m == MC - 1),
                    )
                osb = opool.tile([128, FEAT], F32, tag="osb")
                nc.vector.tensor_add(osb, pu, bo_bc)
                nc.sync.dma_start(
                    out=out[b, tok0 + 128 * s : tok0 + 128 * (s + 1), :], in_=osb
                )


def build():
    nc = bacc.Bacc(
        "TRN2", target_bir_lowering=False, debug=False, num_devices=N_CORES
    )
    x = nc.dram_tensor("x", [BP, N, FEAT], F32, kind="ExternalInput").ap()
    cond = nc.dram_tensor("cond", [BP, M, CD], F32, kind="ExternalInput").ap()
    Wq = nc.dram_tensor("Wq", [FEAT, DA], F32, kind="ExternalInput").ap()
    Wk = nc.dram_tensor("Wk", [CD, DA], F32, kind="ExternalInput").ap()
    Wv = nc.dram_tensor("Wv", [CD, DA], F32, kind="ExternalInput").ap()
    Wo = nc.dram_tensor("Wo", [DA, FEAT], F32, kind="ExternalInput").ap()
    bo = nc.dram_tensor("bo", [FEAT], F32, kind="ExternalInput").ap()
    out = nc.dram_tensor("out", [BP, N, FEAT], F32, kind="ExternalOutput").ap()
    x_bf = nc.dram_tensor("x_bf16_stage", [BP, N, FEAT], BF16).ap()
    with tile.TileContext(nc) as tc:
        _body(tc, x, x_bf, cond, Wq, Wk, Wv, Wo, bo, out)
    nc.compile()
    return nc


_NC = None


def kernel(x, cond, Wq, Wk, Wv, Wo, bo, _trace=False):
    global _NC
    if _NC is None:
        _NC = build()
    shared = {
        "Wq": np.asarray(Wq, np.float32),
        "Wk": np.asarray(Wk, np.float32),
        "Wv": np.asarray(Wv, np.float32),
        "Wo": np.asarray(Wo, np.float32),
        "bo": np.asarray(bo, np.float32),
    }
    in_maps = [
        {
            "x": np.ascontiguousarray(x[BP * i : BP * (i + 1)], dtype=np.float32),
            "cond": np.ascontiguousarray(cond[BP * i : BP * (i + 1)], dtype=np.float32),
            **shared,
        }
        for i in range(N_CORES)
    ]
    res = run_bass_kernel_spmd(_NC, in_maps, list(range(N_CORES)), trace=_trace)
    out = np.concatenate([r["out"] for r in res.results], axis=0)
    if _trace:
        kernel.last_exec_time_ns = res.exec_time_ns
        kernel.last_results = res
    return out
